# revision 12
# baseline (speedup 1.0000x reference)
"""MoE (8 experts, top-2) Trainium2 kernel.

Strategy: expert-parallel across the 8 NeuronCores. The gate (a
4096x1024 @ 1024x8 matmul + top-2 + renormalize, ~0.03% of total FLOPs)
is computed on the host in float64; it only produces routing metadata
(per-token expert ids + combine weights). Each core then runs the full
expert MLP for its expert's tokens:

    y_e = relu(x_e @ W1[e] + b1[e]) @ W2[e] + b2[e]

entirely on device in a fused Bass/Tile kernel (both matmuls, relu and
biases). The host scatters  out[t] = sum_e w_te * y_e[t]  back (the
combine weights are zero for non-selected experts, so routed compute is
mathematically identical to the reference's dense compute).

Device layout (per core, transposed activations so biases are
per-partition):
  MM1:  hT[FF, C] = W1.T @ xT   (+b1, relu)   lhsT = W1 k-tiles
  MM2:  yT[H, C]  = W2.T @ hT   (+b2)         lhsT = W2 k-tiles
with C = per-expert token capacity (padded), all accumulation in fp32
PSUM. Weights are streamed through SBUF pools; xT and hT stay
SBUF-resident.

Schedule (v2, from trace analysis of the 146us baseline):
  - The PE p-state ramps (0.65 -> 1.2 -> 2.4 GHz after ~3us of
    continuous busy). A handful of dummy matmuls on a memset tile keep
    the PE busy during the input-DMA cold start, so the real matmul
    stream runs at full clock from its first instruction and the
    (coarsened, sem>=29) first-evict wait is met long before PSUM
    banks need recycling.
  - DMA triggers (~0.65us each on the issuing sequencer) are spread:
    xt k-slices on sync+vector, w1/w2 on scalar, b1/b2 on gpsimd, so
    the first w1 chunk is triggered immediately at preamble end.
  - MM1 relu evicts run on the (otherwise idle) DVE via tensor_scalar
    (add bias, max 0); MM2 evicts on the ACT engine. Neither queues
    behind DMA triggers.
  - y is returned as f16 (half the output DMA bytes; output
    quantization ~5e-4 relative, well within tolerance) and the very
    last MM2 n-tile is only 128 columns so the final evict+DMA tail
    after the last matmul is short.
"""

import numpy as np

# ---------------------------------------------------------------- config
NUM_EXPERTS = 8
TOP_K = 2
B, S, H = 4, 1024, 1024
FF = 2 * H
T = B * S
P = 128
KH = H // P    # 8 k-tiles over H
KF = FF // P   # 16 k-tiles over FF
CAP_ALIGN = 4  # token capacity alignment (moving dim is arbitrary)
NTILE = 512    # max matmul moving free dim (one PSUM bank of fp32)
TAIL_N = 128   # last n-tile of the last MM2 m-tile (short tail)
N_WARM = 12    # PE clock pre-warm matmuls (cover the DMA-bound cold start)
MM_DT_NAME = "f16"  # one of: bf16, f16, f32r, f32

PROFILE = False       # set True (from test.py) to trace + record HW time
LAST_EXEC_NS = None
LAST_RESULTS = None

_cache = {}


def _mm_dt():
    import concourse.mybir as mybir
    import ml_dtypes

    return {
        "bf16": (mybir.dt.bfloat16, ml_dtypes.bfloat16),
        "f16": (mybir.dt.float16, np.float16),
        "f32r": (mybir.dt.float32r, np.float32),
        "f32": (mybir.dt.float32, np.float32),
    }[MM_DT_NAME]


def _equal_tiles(total, max_n):
    """Split `total` into equal-ish chunks each <= max_n (N large enough
    that the matmul moving stream hides the ~97ns LDWEIGHTS)."""
    n_splits = -(-total // max_n)
    base = total // n_splits
    rem = total - base * n_splits
    tiles = []
    n0 = 0
    for j in range(n_splits):
        nsz = base + (1 if j < rem else 0)
        tiles.append((n0, nsz))
        n0 += nsz
    return tiles


def _build(CAP):
    """Build + compile the per-core Bass program (same for all cores)."""
    import concourse.bass as bass
    import concourse.mybir as mybir
    import concourse.tile as tile
    from concourse import bacc

    mm_dt, _ = _mm_dt()
    f32 = mybir.dt.float32
    ts, ds = bass.ts, bass.ds
    add_op = mybir.AluOpType.add
    max_op = mybir.AluOpType.max

    nc = bacc.Bacc("TRN2", debug=False, num_devices=NUM_EXPERTS)

    xt_d = nc.dram_tensor("xt_d", [KH, P, CAP], mm_dt, kind="ExternalInput")
    w1_d = nc.dram_tensor("w1_d", [KF, P, KH * P], mm_dt, kind="ExternalInput")
    b1_d = nc.dram_tensor("b1_d", [P, KF], f32, kind="ExternalInput")
    w2_d = nc.dram_tensor("w2_d", [KH, P, KF * P], mm_dt, kind="ExternalInput")
    b2_d = nc.dram_tensor("b2_d", [P, KH], f32, kind="ExternalInput")
    y_d = nc.dram_tensor("y_d", [KH, P, CAP], mm_dt, kind="ExternalOutput")

    n_tiles = _equal_tiles(CAP, NTILE)
    # last MM2 m-tile ends with a short n-tile so the post-matmul
    # evict+DMA tail is minimal
    if CAP > TAIL_N and TAIL_N >= 128:
        last_tiles = [
            (n0, nsz) for (n0, nsz) in _equal_tiles(CAP - TAIL_N, NTILE)
        ] + [(CAP - TAIL_N, TAIL_N)]
    else:
        last_tiles = n_tiles

    with tile.TileContext(nc) as tc:
        with (
            tc.tile_pool(name="const", bufs=1) as const,
            tc.tile_pool(name="xtp", bufs=1) as xtp,
            tc.tile_pool(name="hp", bufs=1) as hp,
            tc.tile_pool(name="w1p", bufs=2) as w1p,
            tc.tile_pool(name="w2p", bufs=8) as w2p,
            tc.tile_pool(name="yp", bufs=3) as yp,
            tc.tile_pool(name="psp", bufs=8, space="PSUM") as psp,
        ):
            # ---- PE clock pre-warm: dummy matmuls on a zero tile keep
            # the PE continuously busy through the input-DMA cold start
            # so the real stream starts at the full 2.4GHz p-state.
            warm = const.tile([P, P + NTILE], mm_dt)
            nc.vector.memset(warm[:], 0.0)
            # name must be "acc" so it shares the same pool tag (ring)
            # as the real accumulators below
            warm_acc = psp.tile([P, NTILE], f32, name="acc")
            for _ in range(N_WARM):
                nc.tensor.matmul(
                    warm_acc[:],
                    warm[:, 0:P],
                    warm[:, P : P + NTILE],
                    start=True,
                    stop=True,
                    skip_group_check=True,
                )

            # xt k-slices: one tile per slice so the first matmul only
            # waits on slice 0; triggers split across sync+gpsimd
            # sequencers (~0.65us per DMA trigger, serial per queue).
            # DMA triggers are only legal from SP/Activation/gpsimd.
            # parity split: the DMA engines round-robin between active
            # queues, so alternating queues makes the arrival order match
            # the kh consumption order
            xts = []
            for kh in range(KH):
                xk = xtp.tile([P, CAP], mm_dt, tag=f"xt{kh}")
                eng = nc.sync if kh % 2 == 0 else nc.gpsimd
                eng.dma_start(xk[:], xt_d.ap()[kh])
                xts.append(xk)

            # biases + all w2 m-tiles on the gpsimd queue after xt: the
            # DMA queue is drained in order, so these bytes only move
            # after the critical xt slices have landed. w2 m0 is first
            # needed ~58us into the stream; it's fully resident by ~35us.
            b1t = const.tile([P, KF], f32)
            nc.gpsimd.dma_start(b1t[:], b1_d.ap())
            b2t = const.tile([P, KH], f32)
            nc.gpsimd.dma_start(b2t[:], b2_d.ap())
            w2ts = []
            for m in range(KH):
                w2t = w2p.tile([P, KF * P], mm_dt, name="w2t")
                nc.gpsimd.dma_start(w2t[:], w2_d.ap()[m])
                w2ts.append(w2t)

            h = hp.tile([P, KF, CAP], mm_dt)

            # ---- MM1: hT[kf, :] = relu(W1.T @ xT + b1) ----
            # j (n-tile) is the INNERMOST loop with one open PSUM group
            # per n-tile: consecutive matmuls reuse the same stationary
            # w1 k-slice, and xt[kh] is first needed ~0.46us*kh into the
            # stream instead of all of xt within the first 8 matmuls.
            # The cold start is DMA-bound (~2.5MB at ~350GB/s), so this
            # gradual consumption is what lets the stream start early
            # without stalling.
            for kf in range(KF):
                w1t = w1p.tile([P, KH * P], mm_dt)
                nc.scalar.dma_start(w1t[:], w1_d.ap()[kf])
                accs = [psp.tile([P, NTILE], f32, name="acc") for _ in n_tiles]
                for kh in range(KH):
                    for acc, (n0, nsz) in zip(accs, n_tiles):
                        nc.tensor.matmul(
                            acc[:, :nsz],
                            w1t[:, ts(kh, P)],
                            xts[kh][:, ds(n0, nsz)],
                            start=(kh == 0),
                            stop=(kh == KH - 1),
                            skip_group_check=True,
                        )
                for acc, (n0, nsz) in zip(accs, n_tiles):
                    # relu(acc + b1) on the DVE: (acc + bias) max 0
                    nc.vector.tensor_scalar(
                        h[:, kf, ds(n0, nsz)],
                        acc[:, :nsz],
                        b1t[:, kf : kf + 1],
                        0.0,
                        add_op,
                        max_op,
                    )

            # ---- MM2: yT[m, :] = W2.T @ hT + b2 ----
            for m in range(KH):
                w2t = w2ts[m]
                m_tiles = last_tiles if m == KH - 1 else n_tiles
                for (n0, nsz) in m_tiles:
                    acc = psp.tile([P, NTILE], f32)
                    for k in range(KF):
                        nc.tensor.matmul(
                            acc[:, :nsz],
                            w2t[:, ts(k, P)],
                            h[:, k, ds(n0, nsz)],
                            start=(k == 0),
                            stop=(k == KF - 1),
                            skip_group_check=True,
                        )
                    yt = yp.tile([P, NTILE], mm_dt)
                    nc.scalar.activation(
                        yt[:, :nsz],
                        acc[:, :nsz],
                        mybir.ActivationFunctionType.Identity,
                        bias=b2t[:, m : m + 1],
                    )
                    nc.sync.dma_start(y_d.ap()[m, :, ds(n0, nsz)], yt[:, :nsz])

    nc.compile()
    return nc


def _install_profile_shim():
    """Make run_bass_kernel_spmd(trace=True) work under axon in this
    container (the boot-time antenv.axon_hooks install is absent)."""
    import contextlib
    import ctypes
    import sys
    import types

    if "antenv.axon_hooks" in sys.modules:
        return
    so_path = "/opt/axon/libaxon_pjrt.so"
    lib = ctypes.CDLL(so_path)
    if not hasattr(lib, "axon_start_nrt_profile"):
        return
    lib.axon_start_nrt_profile.argtypes = [
        ctypes.POINTER(ctypes.c_int64),
        ctypes.c_size_t,
    ]
    lib.axon_start_nrt_profile.restype = ctypes.c_int64
    lib.axon_stop_nrt_profile.argtypes = [ctypes.c_char_p]
    lib.axon_stop_nrt_profile.restype = ctypes.c_int64

    @contextlib.contextmanager
    def _hook(output_dir, device_ids):
        import jax

        jax.devices()
        if device_ids:
            ids = (ctypes.c_int64 * len(device_ids))(*device_ids)
            rc = lib.axon_start_nrt_profile(ids, len(device_ids))
        else:
            rc = lib.axon_start_nrt_profile(None, 0)
        if rc != 0:
            raise RuntimeError(f"axon_start_nrt_profile rc={rc}")
        try:
            yield
        finally:
            n = lib.axon_stop_nrt_profile(str(output_dir).encode())
            print(f"ntff profile: {n} file(s) in {output_dir}", file=sys.stderr)

    mod = types.ModuleType("antenv.axon_hooks")
    mod.get_axon_ntff_profile_hook = lambda: _hook
    mod.set_axon_ntff_profile_hook = lambda h: None
    sys.modules["antenv.axon_hooks"] = mod

    import concourse.bass_utils as bu

    bu.upload_artifacts = lambda tmpdir: str(tmpdir)


# ---------------------------------------------------------------- host side

def _route(xf, Wg, bg):
    """Top-2 routing on host, float64 scoring. Returns (top2 [T,2] int,
    w [T,2] float32 renormalized combine weights)."""
    logits = xf.astype(np.float64) @ Wg.astype(np.float64) + bg.astype(np.float64)
    top2 = np.argsort(-logits, axis=-1, kind="stable")[:, :TOP_K]
    lv = np.take_along_axis(logits, top2, axis=1)
    lv = lv - lv.max(axis=1, keepdims=True)
    ev = np.exp(lv)
    w = ev / ev.sum(axis=1, keepdims=True)
    return top2, w.astype(np.float32)


def _prep_weights(W1, b1, W2, b2, np_dt):
    """Per-expert DRAM layouts for the device program."""
    per_expert = []
    for e in range(NUM_EXPERTS):
        w1g = (
            W1[e]
            .reshape(KH, P, KF, P)
            .transpose(2, 1, 0, 3)
            .reshape(KF, P, KH * P)
            .astype(np_dt)
        )
        w2g = (
            W2[e]
            .reshape(KF, P, KH, P)
            .transpose(2, 1, 0, 3)
            .reshape(KH, P, KF * P)
            .astype(np_dt)
        )
        b1g = np.ascontiguousarray(b1[e].reshape(KF, P).T).astype(np.float32)
        b2g = np.ascontiguousarray(b2[e].reshape(KH, P).T).astype(np.float32)
        per_expert.append((w1g, w2g, b1g, b2g))
    return per_expert


def kernel(x, Wg, bg, W1, b1, W2, b2):
    global LAST_EXEC_NS, LAST_RESULTS

    x = np.asarray(x, dtype=np.float32)
    Wg = np.asarray(Wg, dtype=np.float32)
    bg = np.asarray(bg, dtype=np.float32)
    W1 = np.asarray(W1, dtype=np.float32)
    b1 = np.asarray(b1, dtype=np.float32)
    W2 = np.asarray(W2, dtype=np.float32)
    b2 = np.asarray(b2, dtype=np.float32)

    _, np_dt = _mm_dt()
    if PROFILE:
        _install_profile_shim()

    from concourse.bass_utils import run_bass_kernel_spmd

    xf = x.reshape(T, H)
    top2, w = _route(xf, Wg, bg)

    per_expert = _prep_weights(W1, b1, W2, b2, np_dt)

    # token lists per expert
    idx_list = []
    wgt_list = []
    for e in range(NUM_EXPERTS):
        mask = top2 == e  # [T, 2]
        idx = np.where(mask.any(axis=1))[0]
        slot = mask[idx, 1].astype(np.int64)  # 0 if slot0, 1 if slot1
        idx_list.append(idx)
        wgt_list.append(w[idx, slot])

    out = np.zeros((T, H), dtype=np.float32)
    max_count = max(len(i) for i in idx_list)
    # capacity: fit the hottest expert exactly (aligned), bounded so a
    # pathological distribution falls back to multiple rounds
    CAP = min(2048, max(512, -(-max_count // CAP_ALIGN) * CAP_ALIGN))
    if CAP not in _cache:
        _cache[CAP] = _build(CAP)
    nc = _cache[CAP]
    n_rounds = max(1, -(-max_count // CAP))

    for r in range(n_rounds):
        in_maps = []
        chunk_idx = []
        for e in range(NUM_EXPERTS):
            idx = idx_list[e][r * CAP : (r + 1) * CAP]
            chunk_idx.append(idx)
            c = len(idx)
            xe = np.zeros((H, CAP), dtype=np_dt)
            if c:
                xe[:, :c] = xf[idx].T.astype(np_dt)
            w1g, w2g, b1g, b2g = per_expert[e]
            in_maps.append(
                {
                    "xt_d": xe.reshape(KH, P, CAP),
                    "w1_d": w1g,
                    "b1_d": b1g,
                    "w2_d": w2g,
                    "b2_d": b2g,
                }
            )
        res = run_bass_kernel_spmd(
            nc,
            in_maps,
            core_ids=list(range(NUM_EXPERTS)),
            trace=bool(PROFILE),
        )
        if PROFILE:
            LAST_EXEC_NS = res.exec_time_ns
            LAST_RESULTS = res
        for e in range(NUM_EXPERTS):
            idx = chunk_idx[e]
            c = len(idx)
            if not c:
                continue
            yT = res.results[e]["y_d"].reshape(H, CAP).astype(np.float32)
            we = wgt_list[e][r * CAP : (r + 1) * CAP]
            out[idx] += we[:, None] * yT[:, :c].T

    return out.reshape(B, S, H)


# revision 17
# speedup vs baseline: 1.0665x; 1.0665x over previous
"""MoE (8 experts, top-2) Trainium2 kernel.

Strategy: expert-parallel across the 8 NeuronCores. The gate (a
4096x1024 @ 1024x8 matmul + top-2 + renormalize, ~0.03% of total FLOPs)
is computed on the host in float64; it only produces routing metadata
(per-token expert ids + combine weights). Each core then runs the full
expert MLP for its expert's tokens:

    y_e = relu(x_e @ W1[e] + b1[e]) @ W2[e] + b2[e]

entirely on device in a fused Bass/Tile kernel (both matmuls, relu and
biases). The host scatters  out[t] = sum_e w_te * y_e[t]  back (the
combine weights are zero for non-selected experts, so routed compute is
mathematically identical to the reference's dense compute).

Device layout (per core, transposed activations so biases are
per-partition):
  MM1:  hT[FF, C] = W1.T @ xT   (+b1, relu)   lhsT = W1 k-tiles
  MM2:  yT[H, C]  = W2.T @ hT   (+b2)         lhsT = W2 k-tiles
with C = per-expert token capacity (padded), all accumulation in fp32
PSUM. Weights are streamed through SBUF pools; xT and hT stay
SBUF-resident.

Schedule (v2, from trace analysis of the 146us baseline):
  - The PE p-state ramps (0.65 -> 1.2 -> 2.4 GHz after ~3us of
    continuous busy). A handful of dummy matmuls on a memset tile keep
    the PE busy during the input-DMA cold start, so the real matmul
    stream runs at full clock from its first instruction and the
    (coarsened, sem>=29) first-evict wait is met long before PSUM
    banks need recycling.
  - DMA triggers (~0.65us each on the issuing sequencer) are spread:
    xt k-slices on sync+vector, w1/w2 on scalar, b1/b2 on gpsimd, so
    the first w1 chunk is triggered immediately at preamble end.
  - MM1 relu evicts run on the (otherwise idle) DVE via tensor_scalar
    (add bias, max 0); MM2 evicts on the ACT engine. Neither queues
    behind DMA triggers.
  - y is returned as f16 (half the output DMA bytes; output
    quantization ~5e-4 relative, well within tolerance) and the very
    last MM2 n-tile is only 128 columns so the final evict+DMA tail
    after the last matmul is short.
"""

import numpy as np

# ---------------------------------------------------------------- config
NUM_EXPERTS = 8
TOP_K = 2
B, S, H = 4, 1024, 1024
FF = 2 * H
T = B * S
P = 128
KH = H // P    # 8 k-tiles over H
KF = FF // P   # 16 k-tiles over FF
CAP_ALIGN = 4  # token capacity alignment (moving dim is arbitrary)
NTILE = 512    # max matmul moving free dim (one PSUM bank of fp32)
TAIL_N = 128   # last n-tile of the last MM2 m-tile (short tail)
N_WARM = 10    # PE clock pre-warm matmuls (cover the DMA-bound cold start)
SPLIT_KF = 2   # first kf chunks run in two half-K phases (xt arrives late)
MM_DT_NAME = "f16"  # one of: bf16, f16, f32r, f32

PROFILE = False       # set True (from test.py) to trace + record HW time
LAST_EXEC_NS = None
LAST_RESULTS = None

_cache = {}


def _mm_dt():
    import concourse.mybir as mybir
    import ml_dtypes

    return {
        "bf16": (mybir.dt.bfloat16, ml_dtypes.bfloat16),
        "f16": (mybir.dt.float16, np.float16),
        "f32r": (mybir.dt.float32r, np.float32),
        "f32": (mybir.dt.float32, np.float32),
    }[MM_DT_NAME]


def _equal_tiles(total, max_n):
    """Split `total` into equal-ish chunks each <= max_n (N large enough
    that the matmul moving stream hides the ~97ns LDWEIGHTS)."""
    n_splits = -(-total // max_n)
    base = total // n_splits
    rem = total - base * n_splits
    tiles = []
    n0 = 0
    for j in range(n_splits):
        nsz = base + (1 if j < rem else 0)
        tiles.append((n0, nsz))
        n0 += nsz
    return tiles


def _build(CAP):
    """Build + compile the per-core Bass program (same for all cores)."""
    import concourse.bass as bass
    import concourse.mybir as mybir
    import concourse.tile as tile
    from concourse import bacc

    mm_dt, _ = _mm_dt()
    f32 = mybir.dt.float32
    ts, ds = bass.ts, bass.ds
    add_op = mybir.AluOpType.add
    max_op = mybir.AluOpType.max

    nc = bacc.Bacc("TRN2", debug=False, num_devices=NUM_EXPERTS)

    xt_d = nc.dram_tensor("xt_d", [KH, P, CAP], mm_dt, kind="ExternalInput")
    w1_d = nc.dram_tensor("w1_d", [KF, P, KH * P], mm_dt, kind="ExternalInput")
    b1_d = nc.dram_tensor("b1_d", [P, KF], f32, kind="ExternalInput")
    w2_d = nc.dram_tensor("w2_d", [KH, P, KF * P], mm_dt, kind="ExternalInput")
    b2_d = nc.dram_tensor("b2_d", [P, KH], f32, kind="ExternalInput")
    y_d = nc.dram_tensor("y_d", [KH, P, CAP], mm_dt, kind="ExternalOutput")

    n_tiles = _equal_tiles(CAP, NTILE)
    # last MM2 m-tile ends with a short n-tile so the post-matmul
    # evict+DMA tail is minimal
    if CAP > TAIL_N and TAIL_N >= 128:
        last_tiles = [
            (n0, nsz) for (n0, nsz) in _equal_tiles(CAP - TAIL_N, NTILE)
        ] + [(CAP - TAIL_N, TAIL_N)]
    else:
        last_tiles = n_tiles

    with tile.TileContext(nc) as tc:
        with (
            tc.tile_pool(name="const", bufs=1) as const,
            tc.tile_pool(name="xtp", bufs=1) as xtp,
            tc.tile_pool(name="hp", bufs=1) as hp,
            tc.tile_pool(name="w1p", bufs=8) as w1p,
            tc.tile_pool(name="w2p", bufs=8) as w2p,
            tc.tile_pool(name="yp", bufs=3) as yp,
            tc.tile_pool(name="psp", bufs=8, space="PSUM") as psp,
        ):
            # ---- PE clock pre-warm: dummy matmuls on a zero tile keep
            # the PE continuously busy through the input-DMA cold start
            # so the real stream starts at the full 2.4GHz p-state.
            warm = const.tile([P, P + NTILE], mm_dt)
            nc.vector.memset(warm[:], 0.0)
            # name must be "acc" so it shares the same pool tag (ring)
            # as the real accumulators below
            warm_acc = psp.tile([P, NTILE], f32, name="acc")
            for _ in range(N_WARM):
                nc.tensor.matmul(
                    warm_acc[:],
                    warm[:, 0:P],
                    warm[:, P : P + NTILE],
                    start=True,
                    stop=True,
                    skip_group_check=True,
                )

            # xt k-slices: one tile per slice so the first matmul only
            # waits on slice 0; triggers split across sync+gpsimd
            # sequencers (~0.65us per DMA trigger, serial per queue).
            # DMA triggers are only legal from SP/Activation/gpsimd.
            # parity split: the DMA engines round-robin between active
            # queues, so alternating queues makes the arrival order match
            # the kh consumption order
            xts = []
            for kh in range(KH):
                xk = xtp.tile([P, CAP], mm_dt, tag=f"xt{kh}")
                eng = nc.sync if kh % 2 == 0 else nc.gpsimd
                eng.dma_start(xk[:], xt_d.ap()[kh])
                xts.append(xk)

            # biases on the gpsimd queue after xt (first needed by the
            # first MM1 evict, well after the stream starts)
            b1t = const.tile([P, KF], f32)
            nc.gpsimd.dma_start(b1t[:], b1_d.ap())
            b2t = const.tile([P, KH], f32)
            nc.gpsimd.dma_start(b2t[:], b2_d.ap())

            h = hp.tile([P, KF, CAP], mm_dt)

            # ---- MM1: hT[kf, :] = relu(W1.T @ xT + b1) ----
            # j (n-tile) is the INNERMOST loop with one open PSUM group
            # per n-tile: consecutive matmuls reuse the same stationary
            # w1 k-slice, and xt[kh] is needed gradually instead of all
            # within the first 8 matmuls. The first SPLIT_KF chunks are
            # additionally emitted in two half-K phases (groups stay
            # open across the phases): the A phases only touch
            # xt[0:KH/2], moving the xt[KH/2:] deadline from stream
            # start +1.2us to +3.7..5.5us — the cold start is
            # DMA-bandwidth-bound (~2.5MB at ~330GB/s), so this is what
            # the xt arrival rate can actually meet.
            KH2 = KH // 2

            def mm1_span(kf, w1t, accs, kh_range, start_kh, stop_kh):
                for kh in kh_range:
                    for acc, (n0, nsz) in zip(accs, n_tiles):
                        nc.tensor.matmul(
                            acc[:, :nsz],
                            w1t[:, ts(kh, P)],
                            xts[kh][:, ds(n0, nsz)],
                            start=(kh == start_kh),
                            stop=(kh == stop_kh),
                            skip_group_check=True,
                        )

            def mm1_evict(kf, accs):
                for acc, (n0, nsz) in zip(accs, n_tiles):
                    # relu(acc + b1) on the DVE: (acc + bias) max 0
                    nc.vector.tensor_scalar(
                        h[:, kf, ds(n0, nsz)],
                        acc[:, :nsz],
                        b1t[:, kf : kf + 1],
                        0.0,
                        add_op,
                        max_op,
                    )

            split_state = []
            for kf in range(SPLIT_KF):
                w1t = w1p.tile([P, KH * P], mm_dt, name="w1t")
                nc.scalar.dma_start(w1t[:], w1_d.ap()[kf])
                accs = [psp.tile([P, NTILE], f32, name="acc") for _ in n_tiles]
                split_state.append((w1t, accs))
                mm1_span(kf, w1t, accs, range(KH2), 0, KH - 1)
            for kf in range(SPLIT_KF):
                w1t, accs = split_state[kf]
                mm1_span(kf, w1t, accs, range(KH2, KH), 0, KH - 1)
                mm1_evict(kf, accs)
            for kf in range(SPLIT_KF, KF):
                w1t = w1p.tile([P, KH * P], mm_dt, name="w1t")
                nc.scalar.dma_start(w1t[:], w1_d.ap()[kf])
                accs = [psp.tile([P, NTILE], f32, name="acc") for _ in n_tiles]
                mm1_span(kf, w1t, accs, range(KH), 0, KH - 1)
                mm1_evict(kf, accs)

            # ---- MM2: yT[m, :] = W2.T @ hT + b2 ----
            for m in range(KH):
                w2t = w2p.tile([P, KF * P], mm_dt, name="w2t")
                nc.scalar.dma_start(w2t[:], w2_d.ap()[m])
                m_tiles = last_tiles if m == KH - 1 else n_tiles
                for (n0, nsz) in m_tiles:
                    acc = psp.tile([P, NTILE], f32)
                    for k in range(KF):
                        nc.tensor.matmul(
                            acc[:, :nsz],
                            w2t[:, ts(k, P)],
                            h[:, k, ds(n0, nsz)],
                            start=(k == 0),
                            stop=(k == KF - 1),
                            skip_group_check=True,
                        )
                    yt = yp.tile([P, NTILE], mm_dt)
                    nc.scalar.activation(
                        yt[:, :nsz],
                        acc[:, :nsz],
                        mybir.ActivationFunctionType.Identity,
                        bias=b2t[:, m : m + 1],
                    )
                    nc.sync.dma_start(y_d.ap()[m, :, ds(n0, nsz)], yt[:, :nsz])

    nc.compile()
    return nc


def _install_profile_shim():
    """Make run_bass_kernel_spmd(trace=True) work under axon in this
    container (the boot-time antenv.axon_hooks install is absent)."""
    import contextlib
    import ctypes
    import sys
    import types

    if "antenv.axon_hooks" in sys.modules:
        return
    so_path = "/opt/axon/libaxon_pjrt.so"
    lib = ctypes.CDLL(so_path)
    if not hasattr(lib, "axon_start_nrt_profile"):
        return
    lib.axon_start_nrt_profile.argtypes = [
        ctypes.POINTER(ctypes.c_int64),
        ctypes.c_size_t,
    ]
    lib.axon_start_nrt_profile.restype = ctypes.c_int64
    lib.axon_stop_nrt_profile.argtypes = [ctypes.c_char_p]
    lib.axon_stop_nrt_profile.restype = ctypes.c_int64

    @contextlib.contextmanager
    def _hook(output_dir, device_ids):
        import jax

        jax.devices()
        if device_ids:
            ids = (ctypes.c_int64 * len(device_ids))(*device_ids)
            rc = lib.axon_start_nrt_profile(ids, len(device_ids))
        else:
            rc = lib.axon_start_nrt_profile(None, 0)
        if rc != 0:
            raise RuntimeError(f"axon_start_nrt_profile rc={rc}")
        try:
            yield
        finally:
            n = lib.axon_stop_nrt_profile(str(output_dir).encode())
            print(f"ntff profile: {n} file(s) in {output_dir}", file=sys.stderr)

    mod = types.ModuleType("antenv.axon_hooks")
    mod.get_axon_ntff_profile_hook = lambda: _hook
    mod.set_axon_ntff_profile_hook = lambda h: None
    sys.modules["antenv.axon_hooks"] = mod

    import concourse.bass_utils as bu

    bu.upload_artifacts = lambda tmpdir: str(tmpdir)


# ---------------------------------------------------------------- host side

def _route(xf, Wg, bg):
    """Top-2 routing on host, float64 scoring. Returns (top2 [T,2] int,
    w [T,2] float32 renormalized combine weights)."""
    logits = xf.astype(np.float64) @ Wg.astype(np.float64) + bg.astype(np.float64)
    top2 = np.argsort(-logits, axis=-1, kind="stable")[:, :TOP_K]
    lv = np.take_along_axis(logits, top2, axis=1)
    lv = lv - lv.max(axis=1, keepdims=True)
    ev = np.exp(lv)
    w = ev / ev.sum(axis=1, keepdims=True)
    return top2, w.astype(np.float32)


def _prep_weights(W1, b1, W2, b2, np_dt):
    """Per-expert DRAM layouts for the device program."""
    per_expert = []
    for e in range(NUM_EXPERTS):
        w1g = (
            W1[e]
            .reshape(KH, P, KF, P)
            .transpose(2, 1, 0, 3)
            .reshape(KF, P, KH * P)
            .astype(np_dt)
        )
        w2g = (
            W2[e]
            .reshape(KF, P, KH, P)
            .transpose(2, 1, 0, 3)
            .reshape(KH, P, KF * P)
            .astype(np_dt)
        )
        b1g = np.ascontiguousarray(b1[e].reshape(KF, P).T).astype(np.float32)
        b2g = np.ascontiguousarray(b2[e].reshape(KH, P).T).astype(np.float32)
        per_expert.append((w1g, w2g, b1g, b2g))
    return per_expert


def kernel(x, Wg, bg, W1, b1, W2, b2):
    global LAST_EXEC_NS, LAST_RESULTS

    x = np.asarray(x, dtype=np.float32)
    Wg = np.asarray(Wg, dtype=np.float32)
    bg = np.asarray(bg, dtype=np.float32)
    W1 = np.asarray(W1, dtype=np.float32)
    b1 = np.asarray(b1, dtype=np.float32)
    W2 = np.asarray(W2, dtype=np.float32)
    b2 = np.asarray(b2, dtype=np.float32)

    _, np_dt = _mm_dt()
    if PROFILE:
        _install_profile_shim()

    from concourse.bass_utils import run_bass_kernel_spmd

    xf = x.reshape(T, H)
    top2, w = _route(xf, Wg, bg)

    per_expert = _prep_weights(W1, b1, W2, b2, np_dt)

    # token lists per expert
    idx_list = []
    wgt_list = []
    for e in range(NUM_EXPERTS):
        mask = top2 == e  # [T, 2]
        idx = np.where(mask.any(axis=1))[0]
        slot = mask[idx, 1].astype(np.int64)  # 0 if slot0, 1 if slot1
        idx_list.append(idx)
        wgt_list.append(w[idx, slot])

    out = np.zeros((T, H), dtype=np.float32)
    max_count = max(len(i) for i in idx_list)
    # capacity: fit the hottest expert exactly (aligned), bounded so a
    # pathological distribution falls back to multiple rounds
    CAP = min(2048, max(512, -(-max_count // CAP_ALIGN) * CAP_ALIGN))
    if CAP not in _cache:
        _cache[CAP] = _build(CAP)
    nc = _cache[CAP]
    n_rounds = max(1, -(-max_count // CAP))

    for r in range(n_rounds):
        in_maps = []
        chunk_idx = []
        for e in range(NUM_EXPERTS):
            idx = idx_list[e][r * CAP : (r + 1) * CAP]
            chunk_idx.append(idx)
            c = len(idx)
            xe = np.zeros((H, CAP), dtype=np_dt)
            if c:
                xe[:, :c] = xf[idx].T.astype(np_dt)
            w1g, w2g, b1g, b2g = per_expert[e]
            in_maps.append(
                {
                    "xt_d": xe.reshape(KH, P, CAP),
                    "w1_d": w1g,
                    "b1_d": b1g,
                    "w2_d": w2g,
                    "b2_d": b2g,
                }
            )
        res = run_bass_kernel_spmd(
            nc,
            in_maps,
            core_ids=list(range(NUM_EXPERTS)),
            trace=bool(PROFILE),
        )
        if PROFILE:
            LAST_EXEC_NS = res.exec_time_ns
            LAST_RESULTS = res
        for e in range(NUM_EXPERTS):
            idx = chunk_idx[e]
            c = len(idx)
            if not c:
                continue
            yT = res.results[e]["y_d"].reshape(H, CAP).astype(np.float32)
            we = wgt_list[e][r * CAP : (r + 1) * CAP]
            out[idx] += we[:, None] * yT[:, :c].T

    return out.reshape(B, S, H)
